# revision 5
# baseline (speedup 1.0000x reference)
"""Distributed sparse-attention kernel for Trainium2 (8 NeuronCores).

Reference computation (single device):
    q = W @ query + b                  # [512]
    scores = key @ q                   # [262144]
    weight = softmax(scores)           # over all N
    out = weight @ value               # [512]

Strategy: shard key/value row-wise (N) across 8 cores. Each core:
  - computes q = W @ query + b (replicated, tiny)
  - streams its 32768 key rows, computing local scores with fused
    multiply-reduce on the vector engine
  - takes a local max, exponentiates (scalar engine, fused exp-sum)
  - streams its 32768 value rows through the tensor engine,
    accumulating sum_n exp(s_n - m_local) * v_n into PSUM
  - outputs (U_local [512], m_local, s_local)
Host combines the 8 partial results (standard log-sum-exp merge).
"""

import numpy as np

import concourse.bacc as bacc
import concourse.tile as tile
from concourse import mybir
from concourse.bass_utils import run_bass_kernel_spmd

NCORES = 8
N = 262144
D = 512          # KDIM == vdim
QDIM = 256
NLOC = N // NCORES          # 32768 rows per core
P = 128                     # SBUF partitions
R = 4                       # rows per partition per streamed tile
FD = R * D                  # 2048 f32 = 8KB per partition, 1MB per tile
TILES = NLOC // (P * R)     # 64 tiles per tensor per core
COLS = NLOC // P            # 256 score columns in SBUF

F32 = mybir.dt.float32
AX = mybir.AxisListType
ALU = mybir.AluOpType
ACTF = mybir.ActivationFunctionType


def _build_program():
    import concourse.bass_isa as bass_isa

    nc = bacc.Bacc(
        "TRN2",
        target_bir_lowering=False,
        debug=False,
        enable_asserts=False,
        num_devices=NCORES,
    )

    key = nc.dram_tensor("key_shard", [NLOC, D], F32, kind="ExternalInput")
    value = nc.dram_tensor("value_shard", [NLOC, D], F32, kind="ExternalInput")
    query = nc.dram_tensor("query", [QDIM], F32, kind="ExternalInput")
    W = nc.dram_tensor("W", [D, QDIM], F32, kind="ExternalInput")
    b = nc.dram_tensor("b", [D], F32, kind="ExternalInput")

    out_u = nc.dram_tensor("out_u", [D], F32, kind="ExternalOutput")
    out_m = nc.dram_tensor("out_m", [1], F32, kind="ExternalOutput")
    out_s = nc.dram_tensor("out_s", [1], F32, kind="ExternalOutput")

    q_dram = nc.dram_tensor("q_scratch", [D], F32)  # internal staging for q

    with tile.TileContext(nc) as tc:
        with (
            tc.tile_pool(name="singles", bufs=1) as singles,
            tc.tile_pool(name="keyp", bufs=4) as keyp,
            tc.tile_pool(name="valp", bufs=4) as valp,
            tc.tile_pool(name="scratch", bufs=3) as scratchp,
            tc.tile_pool(name="small", bufs=2) as smallp,
            tc.tile_pool(name="psum", bufs=1, space="PSUM") as psump,
        ):
            # ---- q = W @ query + b, laid out as q_cols[p, c] = q[128c + p]
            qb = singles.tile([P, QDIM], F32)
            nc.sync.dma_start(
                out=qb,
                in_=query.ap().rearrange("(u d) -> u d", u=1).to_broadcast([P, QDIM]),
            )
            q_cols = singles.tile([P, R], F32)
            for c in range(R):
                wt = smallp.tile([P, QDIM], F32)
                nc.sync.dma_start(out=wt, in_=W.ap()[P * c : P * (c + 1), :])
                wsc = scratchp.tile([P, QDIM], F32, tag="wsc")
                nc.vector.tensor_mul(wsc, wt, qb)
                nc.vector.tensor_reduce(
                    out=q_cols[:, c : c + 1], in_=wsc, axis=AX.X, op=ALU.add
                )
            b_cols = singles.tile([P, R], F32)
            nc.sync.dma_start(out=b_cols, in_=b.ap().rearrange("(c p) -> p c", p=P))
            nc.vector.tensor_add(q_cols, q_cols, b_cols)
            # round-trip through DRAM to re-layout q as a [1, 512] row,
            # then broadcast it across all 128 partitions
            nc.sync.dma_start(
                out=q_dram.ap().rearrange("(c p) -> p c", p=P), in_=q_cols
            )
            # q replicated R times along the free dim, for one big multiply
            qfull4 = singles.tile([P, FD], F32)
            for j in range(R):
                nc.sync.dma_start(
                    out=qfull4[:, D * j : D * (j + 1)],
                    in_=q_dram.ap().rearrange("(u d) -> u d", u=1).to_broadcast([P, D]),
                )

            # ---- pass 1: local scores.  scores_buf[p, R*t+j] = <key_row, q>
            # for key row (512t + 4p + j) — the natural layout of a
            # contiguous [128, 2048] f32 tile of 512 consecutive rows.
            # VectorE does the elementwise product, ScalarE the segment sums
            # (via activation accum), so the two engines split pass-1 work.
            scores_buf = singles.tile([P, COLS], F32)
            key_t = key.ap().rearrange("(t p r) d -> t p (r d)", p=P, r=R)
            for t in range(TILES):
                kt = keyp.tile([P, FD], F32)
                nc.sync.dma_start(out=kt, in_=key_t[t])
                tmp = scratchp.tile([P, FD], F32, tag="tmp")
                nc.vector.tensor_mul(tmp, kt, qfull4)
                for j in range(R):
                    junk = scratchp.tile([P, D], F32, tag="junk")
                    nc.scalar.activation(
                        out=junk,
                        in_=tmp[:, D * j : D * (j + 1)],
                        func=ACTF.Identity,
                        bias=0.0,
                        scale=1.0,
                        accum_out=scores_buf[:, R * t + j : R * t + j + 1],
                    )

            # ---- local softmax numerators: w = exp(s - m_local)
            pmax = smallp.tile([P, 1], F32)
            nc.vector.tensor_reduce(out=pmax, in_=scores_buf, axis=AX.X, op=ALU.max)
            gmax = singles.tile([P, 1], F32)
            nc.gpsimd.partition_all_reduce(
                gmax, pmax, channels=P, reduce_op=bass_isa.ReduceOp.max
            )
            neg_gmax = singles.tile([P, 1], F32)
            nc.scalar.mul(neg_gmax, gmax, -1.0)
            weights_buf = singles.tile([P, COLS], F32)
            esum_p = smallp.tile([P, 1], F32)
            nc.scalar.activation(
                out=weights_buf,
                in_=scores_buf,
                func=ACTF.Exp,
                bias=neg_gmax[:, 0:1],
                scale=1.0,
                accum_out=esum_p,
            )
            esum = singles.tile([P, 1], F32)
            nc.gpsimd.partition_all_reduce(
                esum, esum_p, channels=P, reduce_op=bass_isa.ReduceOp.add
            )
            nc.sync.dma_start(out=out_m.ap(), in_=gmax[0:1, 0:1])
            nc.sync.dma_start(out=out_s.ap(), in_=esum[0:1, 0:1])

            # ---- pass 2: U = sum_n w_n * value_n via PSUM accumulation.
            # lhsT column c of weights_buf matches value rows (512t + 4p + j)
            # streamed as rhs [128, 512] slices of the same natural layout.
            val_t = value.ap().rearrange("(t p r) d -> t p (r d)", p=P, r=R)
            acc = psump.tile([1, D], F32)
            for t in range(TILES):
                vt = valp.tile([P, FD], F32)
                nc.sync.dma_start(out=vt, in_=val_t[t])
                for j in range(R):
                    nc.tensor.matmul(
                        acc,
                        weights_buf[:, R * t + j : R * t + j + 1],
                        vt[:, D * j : D * (j + 1)],
                        start=(t == 0 and j == 0),
                        stop=(t == TILES - 1 and j == R - 1),
                    )
            out_sb = singles.tile([1, D], F32)
            nc.vector.tensor_copy(out_sb, acc)
            nc.sync.dma_start(out=out_u.ap(), in_=out_sb)

    nc.compile()
    return nc


_NC = None


def _get_program():
    global _NC
    if _NC is None:
        _NC = _build_program()
    return _NC


def _prepare(inputs):
    key = np.asarray(inputs["key"], dtype=np.float32)
    value = np.asarray(inputs["value"], dtype=np.float32)
    query = np.asarray(inputs["query"], dtype=np.float32)
    W = np.asarray(inputs["W"], dtype=np.float32)
    b = np.asarray(inputs["b"], dtype=np.float32)

    in_maps = []
    for i in range(NCORES):
        sl = slice(i * NLOC, (i + 1) * NLOC)
        in_maps.append(
            {
                "key_shard": np.ascontiguousarray(key[sl]),
                "value_shard": np.ascontiguousarray(value[sl]),
                "query": query,
                "W": np.ascontiguousarray(W),
                "b": b,
            }
        )
    return in_maps


def _combine(per_core_results):
    m = np.array(
        [float(r["out_m"][0]) for r in per_core_results], dtype=np.float64
    )
    s = np.array(
        [float(r["out_s"][0]) for r in per_core_results], dtype=np.float64
    )
    U = np.stack([r["out_u"] for r in per_core_results]).astype(np.float64)

    M = m.max()
    alpha = np.exp(m - M)                  # per-core rescale to the global max
    denom = (alpha * s).sum()
    out = (alpha[:, None] * U).sum(axis=0) / denom
    return out.astype(np.float32)


def _run(inputs, trace=False):
    nc = _get_program()
    in_maps = _prepare(inputs)
    res = run_bass_kernel_spmd(nc, in_maps, list(range(NCORES)), trace=trace)
    return _combine(res.results), res


def kernel(**inputs) -> np.ndarray:
    out, _ = _run(inputs, trace=False)
    return out


# revision 7
# speedup vs baseline: 164.8856x; 164.8856x over previous
"""Distributed sparse-attention kernel for Trainium2 (8 NeuronCores).

Reference computation (single device):
    q = W @ query + b                  # [512]
    scores = key @ q                   # [262144]
    weight = softmax(scores)           # over all N
    out = weight @ value               # [512]

Strategy: shard key/value row-wise (N) across 8 cores. Each core:
  - computes q = W @ query + b (replicated, tiny)
  - streams its 32768 key rows, computing local scores with fused
    multiply-reduce on the vector engine
  - takes a local max, exponentiates (scalar engine, fused exp-sum)
  - streams its 32768 value rows through the tensor engine,
    accumulating sum_n exp(s_n - m_local) * v_n into PSUM
  - outputs (U_local [512], m_local, s_local)
Host combines the 8 partial results (standard log-sum-exp merge).
"""

import numpy as np

import concourse.bacc as bacc
import concourse.tile as tile
from concourse import mybir
from concourse.bass_utils import run_bass_kernel_spmd

NCORES = 8
N = 262144
D = 512          # KDIM == vdim
QDIM = 256
NLOC = N // NCORES          # 32768 rows per core
P = 128                     # SBUF partitions
R = 4                       # rows per partition per streamed tile
FD = R * D                  # 2048 f32 = 8KB per partition, 1MB per tile
TILES = NLOC // (P * R)     # 64 tiles per tensor per core
COLS = NLOC // P            # 256 score columns in SBUF

F32 = mybir.dt.float32
AX = mybir.AxisListType
ALU = mybir.AluOpType
ACTF = mybir.ActivationFunctionType


def _build_program(loop_n=1):
    """loop_n > 1 builds a timing variant that repeats the whole kernel
    body on-device (used by test.py to measure per-iteration HW time
    without per-dispatch RPC overhead)."""
    import contextlib

    import concourse.bass_isa as bass_isa

    nc = bacc.Bacc(
        "TRN2",
        target_bir_lowering=False,
        debug=False,
        enable_asserts=False,
        num_devices=NCORES,
    )

    key = nc.dram_tensor("key_shard", [NLOC, D], F32, kind="ExternalInput")
    value = nc.dram_tensor("value_shard", [NLOC, D], F32, kind="ExternalInput")
    query = nc.dram_tensor("query", [QDIM], F32, kind="ExternalInput")
    W = nc.dram_tensor("W", [D, QDIM], F32, kind="ExternalInput")
    b = nc.dram_tensor("b", [D], F32, kind="ExternalInput")

    out_u = nc.dram_tensor("out_u", [D], F32, kind="ExternalOutput")
    out_m = nc.dram_tensor("out_m", [1], F32, kind="ExternalOutput")
    out_s = nc.dram_tensor("out_s", [1], F32, kind="ExternalOutput")

    q_dram = nc.dram_tensor("q_scratch", [D], F32)  # internal staging for q

    with tile.TileContext(nc) as tc:
        with (
            tc.tile_pool(name="singles", bufs=1) as singles,
            tc.tile_pool(name="keyp", bufs=4) as keyp,
            tc.tile_pool(name="valp", bufs=4) as valp,
            tc.tile_pool(name="scratch", bufs=3) as scratchp,
            tc.tile_pool(name="small", bufs=2) as smallp,
            tc.tile_pool(name="psum", bufs=1, space="PSUM") as psump,
            tc.For_i(0, loop_n, 1) if loop_n > 1 else contextlib.nullcontext(),
        ):
            # ---- q = W @ query + b, laid out as q_cols[p, c] = q[128c + p]
            qb = singles.tile([P, QDIM], F32)
            nc.sync.dma_start(
                out=qb,
                in_=query.ap().rearrange("(u d) -> u d", u=1).to_broadcast([P, QDIM]),
            )
            q_cols = singles.tile([P, R], F32)
            for c in range(R):
                wt = smallp.tile([P, QDIM], F32)
                nc.sync.dma_start(out=wt, in_=W.ap()[P * c : P * (c + 1), :])
                wsc = scratchp.tile([P, QDIM], F32, tag="wsc")
                nc.vector.tensor_mul(wsc, wt, qb)
                nc.vector.tensor_reduce(
                    out=q_cols[:, c : c + 1], in_=wsc, axis=AX.X, op=ALU.add
                )
            b_cols = singles.tile([P, R], F32)
            nc.sync.dma_start(out=b_cols, in_=b.ap().rearrange("(c p) -> p c", p=P))
            nc.vector.tensor_add(q_cols, q_cols, b_cols)
            # round-trip through DRAM to re-layout q as a [1, 512] row,
            # then broadcast it across all 128 partitions
            nc.sync.dma_start(
                out=q_dram.ap().rearrange("(c p) -> p c", p=P), in_=q_cols
            )
            # q replicated R times along the free dim, for one big multiply
            qfull4 = singles.tile([P, FD], F32)
            for j in range(R):
                nc.sync.dma_start(
                    out=qfull4[:, D * j : D * (j + 1)],
                    in_=q_dram.ap().rearrange("(u d) -> u d", u=1).to_broadcast([P, D]),
                )

            # ---- pass 1: local scores.  scores_buf[p, R*t+j] = <key_row, q>
            # for key row (512t + 4p + j) — the natural layout of a
            # contiguous [128, 2048] f32 tile of 512 consecutive rows.
            # VectorE does the elementwise product, ScalarE the segment sums
            # (via activation accum), so the two engines split pass-1 work.
            scores_buf = singles.tile([P, COLS], F32)
            key_t = key.ap().rearrange("(t p r) d -> t p (r d)", p=P, r=R)
            for t in range(TILES):
                kt = keyp.tile([P, FD], F32)
                nc.sync.dma_start(out=kt, in_=key_t[t])
                tmp = scratchp.tile([P, FD], F32, tag="tmp")
                nc.vector.tensor_mul(tmp, kt, qfull4)
                for j in range(R):
                    junk = scratchp.tile([P, D], F32, tag="junk")
                    nc.scalar.activation(
                        out=junk,
                        in_=tmp[:, D * j : D * (j + 1)],
                        func=ACTF.Identity,
                        bias=0.0,
                        scale=1.0,
                        accum_out=scores_buf[:, R * t + j : R * t + j + 1],
                    )

            # ---- local softmax numerators: w = exp(s - m_local)
            pmax = smallp.tile([P, 1], F32)
            nc.vector.tensor_reduce(out=pmax, in_=scores_buf, axis=AX.X, op=ALU.max)
            gmax = singles.tile([P, 1], F32)
            nc.gpsimd.partition_all_reduce(
                gmax, pmax, channels=P, reduce_op=bass_isa.ReduceOp.max
            )
            neg_gmax = singles.tile([P, 1], F32)
            nc.scalar.mul(neg_gmax, gmax, -1.0)
            weights_buf = singles.tile([P, COLS], F32)
            esum_p = smallp.tile([P, 1], F32)
            nc.scalar.activation(
                out=weights_buf,
                in_=scores_buf,
                func=ACTF.Exp,
                bias=neg_gmax[:, 0:1],
                scale=1.0,
                accum_out=esum_p,
            )
            esum = singles.tile([P, 1], F32)
            nc.gpsimd.partition_all_reduce(
                esum, esum_p, channels=P, reduce_op=bass_isa.ReduceOp.add
            )
            nc.sync.dma_start(out=out_m.ap(), in_=gmax[0:1, 0:1])
            nc.sync.dma_start(out=out_s.ap(), in_=esum[0:1, 0:1])

            # ---- pass 2: U = sum_n w_n * value_n via PSUM accumulation.
            # lhsT column c of weights_buf matches value rows (512t + 4p + j)
            # streamed as rhs [128, 512] slices of the same natural layout.
            val_t = value.ap().rearrange("(t p r) d -> t p (r d)", p=P, r=R)
            acc = psump.tile([1, D], F32)
            for t in range(TILES):
                vt = valp.tile([P, FD], F32)
                nc.sync.dma_start(out=vt, in_=val_t[t])
                for j in range(R):
                    nc.tensor.matmul(
                        acc,
                        weights_buf[:, R * t + j : R * t + j + 1],
                        vt[:, D * j : D * (j + 1)],
                        start=(t == 0 and j == 0),
                        stop=(t == TILES - 1 and j == R - 1),
                    )
            out_sb = singles.tile([1, D], F32)
            nc.vector.tensor_copy(out_sb, acc)
            nc.sync.dma_start(out=out_u.ap(), in_=out_sb)

    nc.compile()
    return nc


_NC = None


def _get_program():
    global _NC
    if _NC is None:
        _NC = _build_program()
    return _NC


def _prepare(inputs):
    key = np.asarray(inputs["key"], dtype=np.float32)
    value = np.asarray(inputs["value"], dtype=np.float32)
    query = np.asarray(inputs["query"], dtype=np.float32)
    W = np.asarray(inputs["W"], dtype=np.float32)
    b = np.asarray(inputs["b"], dtype=np.float32)

    in_maps = []
    for i in range(NCORES):
        sl = slice(i * NLOC, (i + 1) * NLOC)
        in_maps.append(
            {
                "key_shard": np.ascontiguousarray(key[sl]),
                "value_shard": np.ascontiguousarray(value[sl]),
                "query": query,
                "W": np.ascontiguousarray(W),
                "b": b,
            }
        )
    return in_maps


def _combine(per_core_results):
    m = np.array(
        [float(r["out_m"][0]) for r in per_core_results], dtype=np.float64
    )
    s = np.array(
        [float(r["out_s"][0]) for r in per_core_results], dtype=np.float64
    )
    U = np.stack([r["out_u"] for r in per_core_results]).astype(np.float64)

    M = m.max()
    alpha = np.exp(m - M)                  # per-core rescale to the global max
    denom = (alpha * s).sum()
    out = (alpha[:, None] * U).sum(axis=0) / denom
    return out.astype(np.float32)


def _run(inputs, trace=False):
    nc = _get_program()
    in_maps = _prepare(inputs)
    res = run_bass_kernel_spmd(nc, in_maps, list(range(NCORES)), trace=trace)
    return _combine(res.results), res


def kernel(**inputs) -> np.ndarray:
    out, _ = _run(inputs, trace=False)
    return out


# revision 9
# speedup vs baseline: 166.7994x; 1.0116x over previous
"""Distributed sparse-attention kernel for Trainium2 (8 NeuronCores).

Reference computation (single device):
    q = W @ query + b                  # [512]
    scores = key @ q                   # [262144]
    weight = softmax(scores)           # over all N
    out = weight @ value               # [512]

Strategy: shard key/value row-wise (N) across 8 cores. Each core:
  - computes q = W @ query + b (replicated, tiny)
  - streams its 32768 key rows, computing local scores with fused
    multiply-reduce on the vector engine
  - takes a local max, exponentiates (scalar engine, fused exp-sum)
  - streams its 32768 value rows through the tensor engine,
    accumulating sum_n exp(s_n - m_local) * v_n into PSUM
  - outputs (U_local [512], m_local, s_local)
Host combines the 8 partial results (standard log-sum-exp merge).
"""

import numpy as np

import concourse.bacc as bacc
import concourse.tile as tile
from concourse import mybir
from concourse.bass_utils import run_bass_kernel_spmd

NCORES = 8
N = 262144
D = 512          # KDIM == vdim
QDIM = 256
NLOC = N // NCORES          # 32768 rows per core
P = 128                     # SBUF partitions
R = 4                       # rows per partition per streamed tile
FD = R * D                  # 2048 f32 = 8KB per partition, 1MB per tile
TILES = NLOC // (P * R)     # 64 tiles per tensor per core
COLS = NLOC // P            # 256 score columns in SBUF

F32 = mybir.dt.float32
AX = mybir.AxisListType
ALU = mybir.AluOpType
ACTF = mybir.ActivationFunctionType


def _build_program(loop_n=1):
    """loop_n > 1 builds a timing variant that repeats the whole kernel
    body on-device (used by test.py to measure per-iteration HW time
    without per-dispatch RPC overhead)."""
    import contextlib

    import concourse.bass_isa as bass_isa

    nc = bacc.Bacc(
        "TRN2",
        target_bir_lowering=False,
        debug=False,
        enable_asserts=False,
        num_devices=NCORES,
    )

    key = nc.dram_tensor("key_shard", [NLOC, D], F32, kind="ExternalInput")
    value = nc.dram_tensor("value_shard", [NLOC, D], F32, kind="ExternalInput")
    query = nc.dram_tensor("query", [QDIM], F32, kind="ExternalInput")
    W = nc.dram_tensor("W", [D, QDIM], F32, kind="ExternalInput")
    b = nc.dram_tensor("b", [D], F32, kind="ExternalInput")

    out_u = nc.dram_tensor("out_u", [D], F32, kind="ExternalOutput")
    out_m = nc.dram_tensor("out_m", [1], F32, kind="ExternalOutput")
    out_s = nc.dram_tensor("out_s", [1], F32, kind="ExternalOutput")

    q_dram = nc.dram_tensor("q_scratch", [D], F32)  # internal staging for q

    with tile.TileContext(nc) as tc:
        with (
            tc.tile_pool(name="singles", bufs=1) as singles,
            tc.tile_pool(name="keyp", bufs=6) as keyp,
            tc.tile_pool(name="valp", bufs=8) as valp,
            tc.tile_pool(name="scratch", bufs=3) as scratchp,
            tc.tile_pool(name="small", bufs=2) as smallp,
            tc.tile_pool(name="psum", bufs=1, space="PSUM") as psump,
            tc.For_i(0, loop_n, 1) if loop_n > 1 else contextlib.nullcontext(),
        ):
            # ---- q = W @ query + b, laid out as q_cols[p, c] = q[128c + p]
            qb = singles.tile([P, QDIM], F32)
            nc.sync.dma_start(
                out=qb,
                in_=query.ap().rearrange("(u d) -> u d", u=1).to_broadcast([P, QDIM]),
            )
            q_cols = singles.tile([P, R], F32)
            for c in range(R):
                wt = smallp.tile([P, QDIM], F32)
                nc.sync.dma_start(out=wt, in_=W.ap()[P * c : P * (c + 1), :])
                wsc = scratchp.tile([P, QDIM], F32, tag="wsc")
                nc.vector.tensor_mul(wsc, wt, qb)
                nc.vector.tensor_reduce(
                    out=q_cols[:, c : c + 1], in_=wsc, axis=AX.X, op=ALU.add
                )
            b_cols = singles.tile([P, R], F32)
            nc.sync.dma_start(out=b_cols, in_=b.ap().rearrange("(c p) -> p c", p=P))
            nc.vector.tensor_add(q_cols, q_cols, b_cols)
            # round-trip through DRAM to re-layout q as a [1, 512] row,
            # then broadcast it across all 128 partitions
            nc.sync.dma_start(
                out=q_dram.ap().rearrange("(c p) -> p c", p=P), in_=q_cols
            )
            # q replicated R times along the free dim, for one big multiply
            qfull4 = singles.tile([P, FD], F32)
            for j in range(R):
                nc.sync.dma_start(
                    out=qfull4[:, D * j : D * (j + 1)],
                    in_=q_dram.ap().rearrange("(u d) -> u d", u=1).to_broadcast([P, D]),
                )

            # ---- pass 1: local scores.  scores_buf[p, R*t+j] = <key_row, q>
            # for key row (512t + 4p + j) — the natural layout of a
            # contiguous [128, 2048] f32 tile of 512 consecutive rows.
            # VectorE does the elementwise product, ScalarE the segment sums
            # (via activation accum), so the two engines split pass-1 work.
            scores_buf = singles.tile([P, COLS], F32)
            key_t = key.ap().rearrange("(t p r) d -> t p (r d)", p=P, r=R)
            for t in range(TILES):
                kt = keyp.tile([P, FD], F32)
                nc.sync.dma_start(out=kt, in_=key_t[t])
                tmp = scratchp.tile([P, FD], F32, tag="tmp")
                nc.vector.tensor_mul(tmp, kt, qfull4)
                for j in range(R):
                    junk = scratchp.tile([P, D], F32, tag="junk")
                    nc.scalar.activation(
                        out=junk,
                        in_=tmp[:, D * j : D * (j + 1)],
                        func=ACTF.Identity,
                        bias=0.0,
                        scale=1.0,
                        accum_out=scores_buf[:, R * t + j : R * t + j + 1],
                    )

            # ---- local softmax numerators: w = exp(s - m_local)
            pmax = smallp.tile([P, 1], F32)
            nc.vector.tensor_reduce(out=pmax, in_=scores_buf, axis=AX.X, op=ALU.max)
            gmax = singles.tile([P, 1], F32)
            nc.gpsimd.partition_all_reduce(
                gmax, pmax, channels=P, reduce_op=bass_isa.ReduceOp.max
            )
            neg_gmax = singles.tile([P, 1], F32)
            nc.scalar.mul(neg_gmax, gmax, -1.0)
            weights_buf = singles.tile([P, COLS], F32)
            esum_p = smallp.tile([P, 1], F32)
            nc.scalar.activation(
                out=weights_buf,
                in_=scores_buf,
                func=ACTF.Exp,
                bias=neg_gmax[:, 0:1],
                scale=1.0,
                accum_out=esum_p,
            )
            esum = singles.tile([P, 1], F32)
            nc.gpsimd.partition_all_reduce(
                esum, esum_p, channels=P, reduce_op=bass_isa.ReduceOp.add
            )
            # stats go out on the gpsimd (SWDGE) ring so they never block
            # the sync-engine ring that streams the value tiles
            nc.gpsimd.dma_start(out=out_m.ap(), in_=gmax[0:1, 0:1])
            nc.gpsimd.dma_start(out=out_s.ap(), in_=esum[0:1, 0:1])

            # ---- pass 2: U = sum_n w_n * value_n via PSUM accumulation.
            # lhsT column c of weights_buf matches value rows (512t + 4p + j)
            # streamed as rhs [128, 512] slices of the same natural layout.
            val_t = value.ap().rearrange("(t p r) d -> t p (r d)", p=P, r=R)
            acc = psump.tile([1, D], F32)
            for t in range(TILES):
                vt = valp.tile([P, FD], F32)
                nc.sync.dma_start(out=vt, in_=val_t[t])
                for j in range(R):
                    nc.tensor.matmul(
                        acc,
                        weights_buf[:, R * t + j : R * t + j + 1],
                        vt[:, D * j : D * (j + 1)],
                        start=(t == 0 and j == 0),
                        stop=(t == TILES - 1 and j == R - 1),
                    )
            out_sb = singles.tile([1, D], F32)
            nc.vector.tensor_copy(out_sb, acc)
            nc.sync.dma_start(out=out_u.ap(), in_=out_sb)

    nc.compile()
    return nc


_NC = None


def _get_program():
    global _NC
    if _NC is None:
        _NC = _build_program()
    return _NC


def _prepare(inputs):
    key = np.asarray(inputs["key"], dtype=np.float32)
    value = np.asarray(inputs["value"], dtype=np.float32)
    query = np.asarray(inputs["query"], dtype=np.float32)
    W = np.asarray(inputs["W"], dtype=np.float32)
    b = np.asarray(inputs["b"], dtype=np.float32)

    in_maps = []
    for i in range(NCORES):
        sl = slice(i * NLOC, (i + 1) * NLOC)
        in_maps.append(
            {
                "key_shard": np.ascontiguousarray(key[sl]),
                "value_shard": np.ascontiguousarray(value[sl]),
                "query": query,
                "W": np.ascontiguousarray(W),
                "b": b,
            }
        )
    return in_maps


def _combine(per_core_results):
    m = np.array(
        [float(r["out_m"][0]) for r in per_core_results], dtype=np.float64
    )
    s = np.array(
        [float(r["out_s"][0]) for r in per_core_results], dtype=np.float64
    )
    U = np.stack([r["out_u"] for r in per_core_results]).astype(np.float64)

    M = m.max()
    alpha = np.exp(m - M)                  # per-core rescale to the global max
    denom = (alpha * s).sum()
    out = (alpha[:, None] * U).sum(axis=0) / denom
    return out.astype(np.float32)


def _run(inputs, trace=False):
    nc = _get_program()
    in_maps = _prepare(inputs)
    res = run_bass_kernel_spmd(nc, in_maps, list(range(NCORES)), trace=trace)
    return _combine(res.results), res


def kernel(**inputs) -> np.ndarray:
    out, _ = _run(inputs, trace=False)
    return out


# revision 11
# speedup vs baseline: 174.4886x; 1.0461x over previous
"""Distributed sparse-attention kernel for Trainium2 (8 NeuronCores).

Reference computation (single device):
    q = W @ query + b                  # [512]
    scores = key @ q                   # [262144]
    weight = softmax(scores)           # over all N
    out = weight @ value               # [512]

Strategy: shard key/value row-wise (N) across 8 cores. Each core:
  - computes q = W @ query + b (replicated, tiny)
  - streams its 32768 key rows, computing local scores with fused
    multiply-reduce on the vector engine
  - takes a local max, exponentiates (scalar engine, fused exp-sum)
  - streams its 32768 value rows through the tensor engine,
    accumulating sum_n exp(s_n - m_local) * v_n into PSUM
  - outputs (U_local [512], m_local, s_local)
Host combines the 8 partial results (standard log-sum-exp merge).
"""

import numpy as np

import concourse.bacc as bacc
import concourse.tile as tile
from concourse import mybir
from concourse.bass_utils import run_bass_kernel_spmd

NCORES = 8
N = 262144
D = 512          # KDIM == vdim
QDIM = 256
NLOC = N // NCORES          # 32768 rows per core
P = 128                     # SBUF partitions
R = 4                       # rows per partition per streamed tile
FD = R * D                  # 2048 f32 = 8KB per partition, 1MB per tile
TILES = NLOC // (P * R)     # 64 tiles per tensor per core
COLS = NLOC // P            # 256 score columns in SBUF

F32 = mybir.dt.float32
AX = mybir.AxisListType
ALU = mybir.AluOpType
ACTF = mybir.ActivationFunctionType


def _build_program(loop_n=1):
    """loop_n > 1 builds a timing variant that repeats the whole kernel
    body on-device (used by test.py to measure per-iteration HW time
    without per-dispatch RPC overhead)."""
    import contextlib

    import concourse.bass_isa as bass_isa

    nc = bacc.Bacc(
        "TRN2",
        target_bir_lowering=False,
        debug=False,
        enable_asserts=False,
        num_devices=NCORES,
    )

    key = nc.dram_tensor("key_shard", [NLOC, D], F32, kind="ExternalInput")
    value = nc.dram_tensor("value_shard", [NLOC, D], F32, kind="ExternalInput")
    query = nc.dram_tensor("query", [QDIM], F32, kind="ExternalInput")
    W = nc.dram_tensor("W", [D, QDIM], F32, kind="ExternalInput")
    b = nc.dram_tensor("b", [D], F32, kind="ExternalInput")

    out_u = nc.dram_tensor("out_u", [D], F32, kind="ExternalOutput")
    out_m = nc.dram_tensor("out_m", [1], F32, kind="ExternalOutput")
    out_s = nc.dram_tensor("out_s", [1], F32, kind="ExternalOutput")

    q_dram = nc.dram_tensor("q_scratch", [D], F32)  # internal staging for q

    with tile.TileContext(nc) as tc:
        with (
            tc.tile_pool(name="singles", bufs=1) as singles,
            tc.tile_pool(name="keyp", bufs=6) as keyp,
            tc.tile_pool(name="valp", bufs=8) as valp,
            tc.tile_pool(name="scratch", bufs=3) as scratchp,
            tc.tile_pool(name="small", bufs=2) as smallp,
            tc.tile_pool(name="psum", bufs=1, space="PSUM") as psump,
            tc.For_i(0, loop_n, 1) if loop_n > 1 else contextlib.nullcontext(),
        ):
            # ---- q = W @ query + b, laid out as q_cols[p, c] = q[128c + p]
            qb = singles.tile([P, QDIM], F32)
            nc.sync.dma_start(
                out=qb,
                in_=query.ap().rearrange("(u d) -> u d", u=1).to_broadcast([P, QDIM]),
            )
            q_cols = singles.tile([P, R], F32)
            for c in range(R):
                wt = smallp.tile([P, QDIM], F32)
                nc.sync.dma_start(out=wt, in_=W.ap()[P * c : P * (c + 1), :])
                wsc = scratchp.tile([P, QDIM], F32, tag="wsc")
                nc.vector.tensor_mul(wsc, wt, qb)
                nc.vector.tensor_reduce(
                    out=q_cols[:, c : c + 1], in_=wsc, axis=AX.X, op=ALU.add
                )
            b_cols = singles.tile([P, R], F32)
            nc.sync.dma_start(out=b_cols, in_=b.ap().rearrange("(c p) -> p c", p=P))
            nc.vector.tensor_add(q_cols, q_cols, b_cols)
            # round-trip through DRAM to re-layout q as a [1, 512] row,
            # then broadcast it across all 128 partitions
            nc.sync.dma_start(
                out=q_dram.ap().rearrange("(c p) -> p c", p=P), in_=q_cols
            )
            # q replicated R times along the free dim, for one big multiply
            qfull4 = singles.tile([P, FD], F32)
            for j in range(R):
                nc.sync.dma_start(
                    out=qfull4[:, D * j : D * (j + 1)],
                    in_=q_dram.ap().rearrange("(u d) -> u d", u=1).to_broadcast([P, D]),
                )

            # ---- pass 1: local scores.  scores_buf[p, R*t+j] = <key_row, q>
            # for key row (512t + 4p + j) — the natural layout of a
            # contiguous [128, 2048] f32 tile of 512 consecutive rows.
            # VectorE does the elementwise product, ScalarE the segment sums
            # (via activation accum), so the two engines split pass-1 work.
            scores_buf = singles.tile([P, COLS], F32)
            key_t = key.ap().rearrange("(t p r) d -> t p (r d)", p=P, r=R)
            for t in range(TILES):
                kt = keyp.tile([P, FD], F32)
                nc.sync.dma_start(out=kt, in_=key_t[t])
                tmp = scratchp.tile([P, FD], F32, tag="tmp")
                nc.vector.tensor_mul(tmp, kt, qfull4)
                # seg 0 reduces on VectorE, segs 1-3 on ScalarE (activation
                # accum) — balances the two engines under the DMA stream rate
                nc.vector.tensor_reduce(
                    out=scores_buf[:, R * t : R * t + 1],
                    in_=tmp[:, 0:D],
                    axis=AX.X,
                    op=ALU.add,
                )
                for j in range(1, R):
                    junk = scratchp.tile([P, D], F32, tag="junk")
                    nc.scalar.activation(
                        out=junk,
                        in_=tmp[:, D * j : D * (j + 1)],
                        func=ACTF.Identity,
                        bias=0.0,
                        scale=1.0,
                        accum_out=scores_buf[:, R * t + j : R * t + j + 1],
                    )

            # ---- local softmax numerators: w = exp(s - m_local)
            pmax = smallp.tile([P, 1], F32)
            nc.vector.tensor_reduce(out=pmax, in_=scores_buf, axis=AX.X, op=ALU.max)
            gmax = singles.tile([P, 1], F32)
            nc.gpsimd.partition_all_reduce(
                gmax, pmax, channels=P, reduce_op=bass_isa.ReduceOp.max
            )
            neg_gmax = singles.tile([P, 1], F32)
            nc.scalar.mul(neg_gmax, gmax, -1.0)
            weights_buf = singles.tile([P, COLS], F32)
            esum_p = smallp.tile([P, 1], F32)
            nc.scalar.activation(
                out=weights_buf,
                in_=scores_buf,
                func=ACTF.Exp,
                bias=neg_gmax[:, 0:1],
                scale=1.0,
                accum_out=esum_p,
            )
            esum = singles.tile([P, 1], F32)
            nc.gpsimd.partition_all_reduce(
                esum, esum_p, channels=P, reduce_op=bass_isa.ReduceOp.add
            )
            # stats go out on the gpsimd (SWDGE) ring so they never block
            # the sync-engine ring that streams the value tiles
            nc.gpsimd.dma_start(out=out_m.ap(), in_=gmax[0:1, 0:1])
            nc.gpsimd.dma_start(out=out_s.ap(), in_=esum[0:1, 0:1])

            # ---- pass 2: U = sum_n w_n * value_n via PSUM accumulation.
            # lhsT column c of weights_buf matches value rows (512t + 4p + j)
            # streamed as rhs [128, 512] slices of the same natural layout.
            # seg 0 of each tile accumulates on VectorE into a [128, 512]
            # per-partition accumulator; segs 1-3 run on the (slower for
            # fp32) tensor engine.  The partition dim of acc128 is folded by
            # a final ones-matmul.
            val_t = value.ap().rearrange("(t p r) d -> t p (r d)", p=P, r=R)
            acc = psump.tile([1, D], F32)
            acc128 = singles.tile([P, D], F32)
            nc.vector.memset(acc128, 0.0)
            ones_col = singles.tile([P, 1], F32)
            nc.vector.memset(ones_col, 1.0)
            for t in range(TILES):
                vt = valp.tile([P, FD], F32)
                nc.sync.dma_start(out=vt, in_=val_t[t])
                tmp2 = scratchp.tile([P, D], F32, tag="tmp2")
                nc.vector.tensor_scalar_mul(
                    tmp2, vt[:, 0:D], weights_buf[:, R * t : R * t + 1]
                )
                nc.vector.tensor_add(acc128, acc128, tmp2)
                for j in range(1, R):
                    nc.tensor.matmul(
                        acc,
                        weights_buf[:, R * t + j : R * t + j + 1],
                        vt[:, D * j : D * (j + 1)],
                        start=(t == 0 and j == 1),
                        stop=False,
                    )
            # fold acc128's partitions into the same PSUM accumulator
            nc.tensor.matmul(acc, ones_col[:], acc128[:], start=False, stop=True)
            out_sb = singles.tile([1, D], F32)
            nc.vector.tensor_copy(out_sb, acc)
            nc.sync.dma_start(out=out_u.ap(), in_=out_sb)

    nc.compile()
    return nc


_NC = None


def _get_program():
    global _NC
    if _NC is None:
        _NC = _build_program()
    return _NC


def _prepare(inputs):
    key = np.asarray(inputs["key"], dtype=np.float32)
    value = np.asarray(inputs["value"], dtype=np.float32)
    query = np.asarray(inputs["query"], dtype=np.float32)
    W = np.asarray(inputs["W"], dtype=np.float32)
    b = np.asarray(inputs["b"], dtype=np.float32)

    in_maps = []
    for i in range(NCORES):
        sl = slice(i * NLOC, (i + 1) * NLOC)
        in_maps.append(
            {
                "key_shard": np.ascontiguousarray(key[sl]),
                "value_shard": np.ascontiguousarray(value[sl]),
                "query": query,
                "W": np.ascontiguousarray(W),
                "b": b,
            }
        )
    return in_maps


def _combine(per_core_results):
    m = np.array(
        [float(r["out_m"][0]) for r in per_core_results], dtype=np.float64
    )
    s = np.array(
        [float(r["out_s"][0]) for r in per_core_results], dtype=np.float64
    )
    U = np.stack([r["out_u"] for r in per_core_results]).astype(np.float64)

    M = m.max()
    alpha = np.exp(m - M)                  # per-core rescale to the global max
    denom = (alpha * s).sum()
    out = (alpha[:, None] * U).sum(axis=0) / denom
    return out.astype(np.float32)


def _run(inputs, trace=False):
    nc = _get_program()
    in_maps = _prepare(inputs)
    res = run_bass_kernel_spmd(nc, in_maps, list(range(NCORES)), trace=trace)
    return _combine(res.results), res


def kernel(**inputs) -> np.ndarray:
    out, _ = _run(inputs, trace=False)
    return out
